# revision 15
# baseline (speedup 1.0000x reference)
"""GATv2 (3 layers, self-loops, segment softmax) on 8 Trainium2 NeuronCores.

v4 strategy: nodes sharded contiguously across 8 cores; non-self-loop edges
routed to the core owning their dst, sorted by dst, packed into 128-dst
blocks x 128-edge chunks (variable chunk count per block, SPMD-max across
cores). Per layer: node transform [xl|xr] = x @ [Wl|Wr] in bf16 on PE
(activations stored transposed so no PE transpose is needed), AllGather of
the bf16 xl table, then per block: one [P,1]-offset indirect DMA per chunk
gathers xl[src] rows (the HW SWDGE lowering only honors one offset per
partition), per-edge xr[dst] comes from a one-hot matmul against the
block-local xr tile, scores on DVE (leaky-relu must be DVE: ACT Lrelu drops
alpha), exp on ACT, and a host-streamed one-hot matmul does the per-block
segment reduction of softmax numerator+denominator in PSUM. Both one-hot
orientations ([q,e] for the xr gather, [e,q] for the scatter) are built on
the host and streamed per block as one interleaved bf16 tensor. Self-loops
never enter the edge stream: the self contribution for dst node p of block
b is computed in the finalize from the block's local xl/xr rows.

Self-contained: hardcodes problem shapes; no sibling imports.
"""
import numpy as np
import ml_dtypes

P = 128          # partitions / block size / chunk size
NEG_SLOPE = 0.2
BF16 = ml_dtypes.bfloat16


# ---------------------------------------------------------------- host prep

def prep_edges(src, dst, N, ncores):
    """Route edges to dst-owning cores, sort by dst, pack into block/chunk slots.

    Returns (Nshard, nblk, chks, ids, oh):
      chks int list [nblk]: chunks used per block (max across cores).
      ids int32 [ncores, nblk, P, CH]: global src id per slot (0 pad).
      oh bf16 [ncores, nblk, P, CH*2*P]: per chunk ch the column band
        [ch*2P : ch*2P+P] is the gather one-hot oh2[q, e] = (rel[e]==q) and
        [ch*2P+P : (ch+1)*2P] is the scatter one-hot ot[e, q] = (rel[e]==q);
        pad slots are all-zero in both.
    """
    Nshard = ((N + ncores * P - 1) // (ncores * P)) * P
    nblk = Nshard // P
    core = dst // Nshard
    percore = []
    cnts = np.zeros((ncores, nblk), np.int64)
    for c in range(ncores):
        m = core == c
        s = src[m]
        dl = (dst[m] - c * Nshard).astype(np.int64)
        o = np.argsort(dl, kind='stable')
        s, dl = s[o], dl[o]
        blk = dl // P
        counts = np.bincount(blk, minlength=nblk)
        cnts[c] = counts
        percore.append((s, dl, blk, counts))
    chks = np.maximum(1, (cnts.max(axis=0) + P - 1) // P)   # [nblk], SPMD max
    CH = int(chks.max())
    ids = np.zeros((ncores, nblk, CH, P), np.int32)
    oh = np.zeros((ncores, nblk, P, CH * 2 * P), BF16)
    for c in range(ncores):
        s, dl, blk, counts = percore[c]
        starts = np.zeros(nblk, np.int64)
        starts[1:] = np.cumsum(counts)[:-1]
        pos = np.arange(len(s)) - starts[blk]        # rank within block
        ch = pos // P
        p = pos % P
        ids[c, blk, ch, p] = s
        rel = (dl - blk * P).astype(np.int64)
        oh[c, blk, rel, ch * 2 * P + p] = 1          # gather: [q=rel, e=p]
        oh[c, blk, p, ch * 2 * P + P + rel] = 1      # scatter: [e=p, q=rel]
    # ids: [c, nblk, CH, P] -> [c, nblk, P, CH]
    ids = ids.transpose(0, 1, 3, 2)
    return Nshard, nblk, [int(v) for v in chks], np.ascontiguousarray(ids), oh


# ---------------------------------------------------------------- bass build

def build_program(ncores, Nshard, nblk, chks, dims_in, H, C, use_collective=True):
    import concourse.bass as bass
    import concourse.mybir as mybir
    from concourse import bacc
    from concourse.tile import TileContext

    D = H * C
    W = D + H
    L = len(dims_in)
    CH = max(chks)
    Np = Nshard * ncores
    f32, i32, bf16 = mybir.dt.float32, mybir.dt.int32, mybir.dt.bfloat16
    AF = mybir.ActivationFunctionType
    OP = mybir.AluOpType

    nc = bacc.Bacc()
    x0T = nc.declare_dram_parameter("x0T", [dims_in[0], Nshard], bf16, isOutput=False)
    ids = nc.declare_dram_parameter("ids", [nblk, P, CH], i32, isOutput=False)
    oh = nc.declare_dram_parameter("oh", [nblk, P, CH * 2 * P], bf16, isOutput=False)
    wparams = []
    for l in range(L):
        din = dims_in[l]
        wparams.append((
            nc.declare_dram_parameter(f"Wlr{l}", [din, 2 * D], bf16, isOutput=False),
            nc.declare_dram_parameter(f"attb{l}", [P, CH * D], bf16, isOutput=False),
            nc.declare_dram_parameter(f"biasb{l}", [P, D], f32, isOutput=False),
        ))
    ident_in = nc.declare_dram_parameter("ident", [P, P], bf16, isOutput=False)
    y = nc.declare_dram_parameter("y", [Nshard, D], f32, isOutput=True)

    xl_sh = nc.dram_tensor("xl_sh", [Nshard, D], bf16)
    xl_full = nc.dram_tensor("xl_full", [Np, D], bf16, addr_space="Shared")
    xmT = [nc.dram_tensor(f"xmT{i}", [D, Nshard], bf16) for i in range(L - 1)]

    with TileContext(nc) as tc:
        with (
            tc.tile_pool(name="const", bufs=1) as cp,
            tc.tile_pool(name="xr_res", bufs=1) as xrp,
            tc.tile_pool(name="node", bufs=3) as np_,
            tc.tile_pool(name="nodeps", bufs=2, space="PSUM") as nps_,
            tc.tile_pool(name="edge", bufs=3) as ep,
            tc.tile_pool(name="gat", bufs=4) as gp,
            tc.tile_pool(name="xrps", bufs=1, space="PSUM") as xps,
            tc.tile_pool(name="blkps", bufs=2, space="PSUM") as blkps,
            tc.tile_pool(name="fin", bufs=2) as fp,
        ):
            ident = cp.tile([P, P], bf16, tag="ident")
            nc.sync.dma_start(out=ident[:], in_=ident_in[:])
            wts = []
            for l in range(L):
                din = dims_in[l]
                wlr = cp.tile([din, 2 * D], bf16, tag=f"wlr{l}")
                nc.sync.dma_start(out=wlr[:], in_=wparams[l][0][:])
                attb = cp.tile([P, CH * D], bf16, tag=f"attb{l}")
                nc.sync.dma_start(out=attb[:], in_=wparams[l][1][:])
                biasb = cp.tile([P, D], f32, tag=f"biasb{l}")
                nc.sync.dma_start(out=biasb[:], in_=wparams[l][2][:])
                wts.append((wlr, attb, biasb))

            for l in range(L):
                din = dims_in[l]
                x_out = y if l == L - 1 else xmT[l]
                wlr, attb, biasb = wts[l]

                # ---- node transform: [xl|xr] = x @ [Wl|Wr]  (bf16, PE)
                xlr_all = []
                for t in range(nblk):
                    xt = np_.tile([din, P], bf16, tag="xt")
                    if l == 0:
                        nc.sync.dma_start(out=xt[:], in_=x0T[:, t * P:(t + 1) * P])
                    else:
                        nc.sync.dma_start(out=xt[:], in_=xmT[l - 1][:, t * P:(t + 1) * P])
                    mm = nps_.tile([P, 2 * D], f32, tag="mm")
                    nc.tensor.matmul(out=mm[:], lhsT=xt[:], rhs=wlr[:], start=True, stop=True)
                    xlr = xrp.tile([P, 2 * D], bf16, tag=f"xlr{t}")
                    nc.scalar.activation(out=xlr[:], in_=mm[:], func=AF.Copy)
                    nc.sync.dma_start(out=xl_sh[t * P:(t + 1) * P, :], in_=xlr[:, 0:D])
                    xlr_all.append(xlr)

                # ---- gather table for xl across all cores
                if use_collective:
                    nc.gpsimd.collective_compute(
                        "AllGather", OP.bypass,
                        replica_groups=[list(range(ncores))],
                        ins=[xl_sh[:]], outs=[xl_full[:]],
                    )
                else:
                    nc.sync.dma_start(out=xl_full[:], in_=xl_sh[:])

                # ---- edge phase
                for blk in range(nblk):
                    chk = chks[blk]
                    idst = gp.tile([P, CH], i32, tag="idst")
                    nc.sync.dma_start(out=idst[:, 0:chk], in_=ids[blk, :, 0:chk])
                    oht = ep.tile([P, CH * 2 * P], bf16, tag="oht")
                    nc.scalar.dma_start(out=oht[:, 0:chk * 2 * P], in_=oh[blk, :, 0:chk * 2 * P])
                    xl_s = gp.tile([P, CH * D], bf16, tag="xls")
                    for ch in range(chk):
                        nc.gpsimd.indirect_dma_start(
                            out=xl_s[:, ch * D:(ch + 1) * D],
                            out_offset=None, in_=xl_full[:],
                            in_offset=bass.IndirectOffsetOnAxis(
                                ap=idst[:, ch:ch + 1], axis=0))
                    xrb = xlr_all[blk]
                    # xr[dst] per chunk via one-hot matmul (PE, no DMA gather)
                    xr_ps = xps.tile([P, CH * D], f32, tag="xrps")
                    for ch in range(chk):
                        nc.tensor.matmul(
                            out=xr_ps[:, ch * D:(ch + 1) * D],
                            lhsT=oht[:, ch * 2 * P:ch * 2 * P + P],
                            rhs=xrb[:, D:2 * D], start=True, stop=True)
                    nd = chk * D
                    g = ep.tile([P, CH * D], bf16, tag="g")
                    nc.vector.tensor_tensor(out=g[:, 0:nd], in0=xl_s[:, 0:nd],
                                            in1=xr_ps[:, 0:nd], op=OP.add)
                    gl = ep.tile([P, CH * D], bf16, tag="gl")
                    # leaky_relu(g) = max(0.2*g, g); must be DVE (ACT Lrelu drops alpha)
                    nc.vector.scalar_tensor_tensor(
                        out=gl[:, 0:nd], in0=g[:, 0:nd], scalar=NEG_SLOPE,
                        in1=g[:, 0:nd], op0=OP.mult, op1=OP.max)
                    ge = ep.tile([P, CH * D], bf16, tag="ge")
                    nc.vector.tensor_tensor(out=ge[:, 0:nd], in0=gl[:, 0:nd],
                                            in1=attb[:, 0:nd], op=OP.mult)
                    e = ep.tile([P, CH * H], f32, tag="e")
                    nc.vector.tensor_reduce(
                        out=e[:, 0:chk * H],
                        in_=ge[:, 0:nd].rearrange("p (sh c) -> p sh c", c=C),
                        axis=mybir.AxisListType.X, op=OP.add)
                    vals = ep.tile([P, CH * W], bf16, tag="vals")
                    vals_v = vals[:].rearrange("p (s w) -> p s w", s=CH)
                    nc.scalar.activation(
                        out=vals_v[:, 0:chk, D:W],
                        in_=e[:, 0:chk * H].rearrange("p (s h) -> p s h", s=chk),
                        func=AF.Exp)
                    nc.vector.tensor_tensor(
                        out=vals_v[:, 0:chk, 0:D].rearrange("p s (h c) -> p s h c", h=H),
                        in0=xl_s[:, 0:nd].rearrange("p (s h c) -> p s h c", s=chk, h=H),
                        in1=vals_v[:, 0:chk, D:W].unsqueeze(3).to_broadcast([P, chk, H, C]),
                        op=OP.mult)
                    ps = blkps.tile([P, W], f32, tag="ps")
                    for ch in range(chk):
                        nc.tensor.matmul(
                            out=ps[:], lhsT=oht[:, ch * 2 * P + P:(ch + 1) * 2 * P],
                            rhs=vals[:, ch * W:(ch + 1) * W],
                            start=(ch == 0), stop=(ch == chk - 1))
                    # ---- finalize block: self-loop term, div by denom, +bias, elu
                    gs = fp.tile([P, D], bf16, tag="gs")
                    nc.vector.tensor_tensor(out=gs[:], in0=xrb[:, 0:D], in1=xrb[:, D:2 * D],
                                            op=OP.add)
                    nc.vector.scalar_tensor_tensor(
                        out=gs[:], in0=gs[:], scalar=NEG_SLOPE, in1=gs[:],
                        op0=OP.mult, op1=OP.max)
                    nc.vector.tensor_tensor(out=gs[:], in0=gs[:], in1=attb[:, 0:D], op=OP.mult)
                    es = fp.tile([P, H], f32, tag="es")
                    nc.vector.tensor_reduce(
                        out=es[:], in_=gs[:].rearrange("p (h c) -> p h c", h=H),
                        axis=mybir.AxisListType.X, op=OP.add)
                    pse = fp.tile([P, H], f32, tag="pse")
                    nc.scalar.activation(out=pse[:], in_=es[:], func=AF.Exp)
                    den = fp.tile([P, H], f32, tag="den")
                    nc.vector.tensor_tensor(out=den[:], in0=ps[:, D:W], in1=pse[:], op=OP.add)
                    r = fp.tile([P, H], f32, tag="r")
                    nc.vector.reciprocal(out=r[:], in_=den[:])
                    num = fp.tile([P, D], f32, tag="num")
                    nc.vector.tensor_tensor(
                        out=num[:].rearrange("p (h c) -> p h c", h=H),
                        in0=xrb[:, 0:D].rearrange("p (h c) -> p h c", h=H),
                        in1=pse[:].unsqueeze(2).to_broadcast([P, H, C]),
                        op=OP.mult)
                    nc.vector.tensor_tensor(out=num[:], in0=num[:], in1=ps[:, 0:D], op=OP.add)
                    o = fp.tile([P, D], f32, tag="o")
                    nc.vector.tensor_tensor(
                        out=o[:].rearrange("p (h c) -> p h c", h=H),
                        in0=num[:].rearrange("p (h c) -> p h c", h=H),
                        in1=r[:].unsqueeze(2).to_broadcast([P, H, C]),
                        op=OP.mult)
                    nc.vector.tensor_tensor(out=o[:], in0=o[:], in1=biasb[:], op=OP.add)
                    t1 = fp.tile([P, D], f32, tag="t1")
                    nc.vector.tensor_scalar(
                        out=t1[:], in0=o[:], scalar1=0.0, scalar2=None, op0=OP.min)
                    nc.scalar.activation(out=t1[:], in_=t1[:], func=AF.Exp)
                    # elu tail: o = max(o, t1 - 1) in one fused DVE op
                    nc.vector.scalar_tensor_tensor(
                        out=o[:], in0=t1[:], scalar=-1.0, in1=o[:],
                        op0=OP.add, op1=OP.max)
                    if l == L - 1:
                        nc.sync.dma_start(out=y[blk * P:(blk + 1) * P, :], in_=o[:])
                    else:
                        ob = fp.tile([P, D], bf16, tag="ob")
                        nc.vector.tensor_copy(out=ob[:], in_=o[:])
                        obT_ps = nps_.tile([D, P], bf16, tag="obT")
                        nc.tensor.transpose(out=obT_ps[:], in_=ob[:], identity=ident[:])
                        obT = fp.tile([D, P], bf16, tag="obTs")
                        nc.scalar.activation(out=obT[:], in_=obT_ps[:], func=AF.Copy)
                        nc.sync.dma_start(out=x_out[:, blk * P:(blk + 1) * P], in_=obT[:])
    nc.compile()
    return nc


# ---------------------------------------------------------------- entry

def make_inmaps(inputs, ncores):
    x = np.asarray(inputs['x'], np.float32)
    ei = np.asarray(inputs['edge_index'], np.int32)
    N, F = x.shape
    H, C = np.asarray(inputs['att0']).shape
    D = H * C
    L = 3
    # the synthetic self-loops (PyG add_self_loops) are handled analytically in
    # the kernel finalize; natural (i,i) edges stay in the edge stream
    src = ei[0]
    dst = ei[1]
    Nshard, nblk, chks, ids, oh = prep_edges(src, dst, N, ncores)
    CH = max(chks)
    xp = np.zeros((Nshard * ncores, F), np.float32)
    xp[:N] = x
    ident = np.eye(P, dtype=BF16)
    dims_in = [F] + [D] * (L - 1)
    base = {"ident": ident}
    for l in range(L):
        wl = np.asarray(inputs[f'Wl{l}'], np.float32)
        wr = np.asarray(inputs[f'Wr{l}'], np.float32)
        base[f"Wlr{l}"] = np.concatenate([wl, wr], axis=1).astype(BF16)
        att = np.asarray(inputs[f'att{l}'], np.float32).reshape(1, D)
        base[f"attb{l}"] = np.broadcast_to(np.tile(att, (1, CH)), (P, CH * D)).astype(BF16)
        b = np.asarray(inputs[f'b{l}'], np.float32).reshape(1, D)
        base[f"biasb{l}"] = np.broadcast_to(b, (P, D)).copy()
    in_maps = []
    for c in range(ncores):
        mm = dict(base)
        mm["x0T"] = np.ascontiguousarray(
            xp[c * Nshard:(c + 1) * Nshard].T).astype(BF16)
        mm["ids"] = np.ascontiguousarray(ids[c])
        mm["oh"] = np.ascontiguousarray(oh[c])
        in_maps.append(mm)
    return in_maps, Nshard, nblk, chks, dims_in, H, C, N, D


def kernel(**inputs):
    from concourse.bass_utils import run_bass_kernel_spmd
    ncores = 8
    in_maps, Nshard, nblk, chks, dims_in, H, C, N, D = make_inmaps(inputs, ncores)
    nc = build_program(ncores, Nshard, nblk, chks, dims_in, H, C, use_collective=True)
    res = run_bass_kernel_spmd(nc, in_maps, list(range(ncores)))
    out = np.concatenate([res.results[c]["y"] for c in range(ncores)], axis=0)
    return out[:N].astype(np.float32)


if __name__ == "__main__":
    pass


# revision 17
# speedup vs baseline: 1.0226x; 1.0226x over previous
"""GATv2 (3 layers, self-loops, segment softmax) on 8 Trainium2 NeuronCores.

v5 strategy: nodes sharded contiguously across 8 cores; non-self-loop edges
routed to the core owning their dst, sorted by dst, packed into 128-dst
blocks x 128-edge chunks (variable chunk count per block, SPMD-max across
cores). Per layer: node transform [xl|xr] = x @ [Wl|Wr] in bf16 on PE
(activations stored transposed; inputs loaded on the tensor queue), the xl
table AllGathered in two halves (first half overlaps the node tail; src row
ids are remapped host-side for the half-interleaved table layout), then per
4-block gather group: one idst load + one [P,1]-offset indirect DMA per
chunk gathers xl[src] rows (HW SWDGE honors one offset per partition; the
grouping cuts per-block Pool-engine semaphore waits 4x). Per block: xr[dst]
via one-hot matmul against the block-local xr tile, scores on DVE
(leaky-relu must be DVE: ACT Lrelu drops alpha), exp on ACT, host-streamed
one-hot matmul for the per-block segment softmax reduction in PSUM.
Self-loops never enter the edge stream: the self contribution for dst node
p of block b is computed in the finalize from the block's local xl/xr rows.

Self-contained: hardcodes problem shapes; no sibling imports.
"""
import numpy as np
import ml_dtypes

P = 128          # partitions / block size / chunk size
G = 4            # blocks per gather group
NEG_SLOPE = 0.2
BF16 = ml_dtypes.bfloat16


# ---------------------------------------------------------------- host prep

def prep_edges(src, dst, N, ncores):
    """Route edges to dst-owning cores, sort by dst, pack into block/chunk slots.

    src ids are remapped for the half-split AllGather table layout:
    node n (core c, local q, half h=q//S2, pos=q%S2) lives at table row
    h*(ncores*S2) + c*S2 + pos, with S2 = Nshard//2.

    Returns (Nshard, nblk, chks, ids, oh):
      chks int list [nblk]: chunks used per block (max across cores).
      ids int32 [ngrp, P, gc]: remapped src id per slot, G blocks per group
        packed along the free axis (0 pad); ngrp = ceil(nblk/G),
        gc = max group column count.
      oh bf16 [ncores, nblk, P, CH*2*P]: per chunk ch the column band
        [ch*2P : ch*2P+P] is the gather one-hot oh2[q, e] = (rel[e]==q) and
        [ch*2P+P : (ch+1)*2P] is the scatter one-hot ot[e, q] = (rel[e]==q);
        pad slots are all-zero in both.
    """
    Nshard = ((N + ncores * P - 1) // (ncores * P)) * P
    nblk = Nshard // P
    S2 = Nshard // 2
    core = dst // Nshard
    percore = []
    cnts = np.zeros((ncores, nblk), np.int64)
    for c in range(ncores):
        m = core == c
        s = src[m]
        dl = (dst[m] - c * Nshard).astype(np.int64)
        o = np.argsort(dl, kind='stable')
        s, dl = s[o], dl[o]
        blk = dl // P
        counts = np.bincount(blk, minlength=nblk)
        cnts[c] = counts
        percore.append((s, dl, blk, counts))
    chks = np.maximum(1, (cnts.max(axis=0) + P - 1) // P)   # [nblk], SPMD max
    CH = int(chks.max())
    ids = np.zeros((ncores, nblk, CH, P), np.int32)
    oh = np.zeros((ncores, nblk, P, CH * 2 * P), BF16)
    for c in range(ncores):
        s, dl, blk, counts = percore[c]
        starts = np.zeros(nblk, np.int64)
        starts[1:] = np.cumsum(counts)[:-1]
        pos = np.arange(len(s)) - starts[blk]        # rank within block
        ch = pos // P
        p = pos % P
        # remap src for half-split AllGather layout
        sc_ = s // Nshard
        sq = s % Nshard
        srow = (sq // S2) * (ncores * S2) + sc_ * S2 + (sq % S2)
        ids[c, blk, ch, p] = srow
        rel = (dl - blk * P).astype(np.int64)
        oh[c, blk, rel, ch * 2 * P + p] = 1          # gather: [q=rel, e=p]
        oh[c, blk, p, ch * 2 * P + P + rel] = 1      # scatter: [e=p, q=rel]
    # pack ids into gather groups: [c, ngrp, P, gc]
    ngrp = (nblk + G - 1) // G
    offs = []        # per group: list of (blk, col offset)
    gc = 0
    for gi in range(ngrp):
        blks = list(range(gi * G, min((gi + 1) * G, nblk)))
        cum = np.concatenate([[0], np.cumsum([chks[b] for b in blks])])
        offs.append((blks, [int(v) for v in cum[:-1]]))
        gc = max(gc, int(cum[-1]))
    idsg = np.zeros((ncores, ngrp, P, gc), np.int32)
    for gi, (blks, co) in enumerate(offs):
        for b, o_ in zip(blks, co):
            k = int(chks[b])
            idsg[:, gi, :, o_:o_ + k] = ids[:, b, :, :].transpose(0, 2, 1)[:, :, :k]
    return (Nshard, nblk, [int(v) for v in chks], offs, gc,
            np.ascontiguousarray(idsg), oh)


# ---------------------------------------------------------------- bass build

def build_program(ncores, Nshard, nblk, chks, offs, gc, dims_in, H, C,
                  use_collective=True):
    import concourse.bass as bass
    import concourse.mybir as mybir
    from concourse import bacc
    from concourse.tile import TileContext

    D = H * C
    W = D + H
    L = len(dims_in)
    CH = max(chks)
    Np = Nshard * ncores
    S2 = Nshard // 2
    ngrp = len(offs)
    f32, i32, bf16 = mybir.dt.float32, mybir.dt.int32, mybir.dt.bfloat16
    AF = mybir.ActivationFunctionType
    OP = mybir.AluOpType

    nc = bacc.Bacc()
    x0T = nc.declare_dram_parameter("x0T", [dims_in[0], Nshard], bf16, isOutput=False)
    ids = nc.declare_dram_parameter("ids", [ngrp, P, gc], i32, isOutput=False)
    oh = nc.declare_dram_parameter("oh", [nblk, P, CH * 2 * P], bf16, isOutput=False)
    wparams = []
    for l in range(L):
        din = dims_in[l]
        wparams.append((
            nc.declare_dram_parameter(f"Wlr{l}", [din, 2 * D], bf16, isOutput=False),
            nc.declare_dram_parameter(f"attb{l}", [P, CH * D], bf16, isOutput=False),
            nc.declare_dram_parameter(f"biasb{l}", [P, D], f32, isOutput=False),
        ))
    ident_in = nc.declare_dram_parameter("ident", [P, P], bf16, isOutput=False)
    y = nc.declare_dram_parameter("y", [Nshard, D], f32, isOutput=True)

    xl_sh = nc.dram_tensor("xl_sh", [Nshard, D], bf16)
    xl_full = nc.dram_tensor("xl_full", [Np, D], bf16, addr_space="Shared")
    xmT = [nc.dram_tensor(f"xmT{i}", [D, Nshard], bf16) for i in range(L - 1)]

    with TileContext(nc) as tc:
        with (
            tc.tile_pool(name="const", bufs=1) as cp,
            tc.tile_pool(name="xr_res", bufs=1) as xrp,
            tc.tile_pool(name="node", bufs=3) as np_,
            tc.tile_pool(name="nodeps", bufs=2, space="PSUM") as nps_,
            tc.tile_pool(name="edge", bufs=3) as ep,
            tc.tile_pool(name="gat", bufs=2) as gp,
            tc.tile_pool(name="xrps", bufs=1, space="PSUM") as xps,
            tc.tile_pool(name="blkps", bufs=2, space="PSUM") as blkps,
            tc.tile_pool(name="fin", bufs=2) as fp,
        ):
            ident = cp.tile([P, P], bf16, tag="ident")
            nc.sync.dma_start(out=ident[:], in_=ident_in[:])
            wts = []
            for l in range(L):
                din = dims_in[l]
                wlr = cp.tile([din, 2 * D], bf16, tag=f"wlr{l}")
                nc.sync.dma_start(out=wlr[:], in_=wparams[l][0][:])
                attb = cp.tile([P, CH * D], bf16, tag=f"attb{l}")
                nc.sync.dma_start(out=attb[:], in_=wparams[l][1][:])
                biasb = cp.tile([P, D], f32, tag=f"biasb{l}")
                nc.sync.dma_start(out=biasb[:], in_=wparams[l][2][:])
                wts.append((wlr, attb, biasb))

            for l in range(L):
                din = dims_in[l]
                x_out = y if l == L - 1 else xmT[l]
                wlr, attb, biasb = wts[l]

                # ---- node transform: [xl|xr] = x @ [Wl|Wr]  (bf16, PE)
                xlr_all = []
                for t in range(nblk):
                    xt = np_.tile([din, P], bf16, tag="xt")
                    if l == 0:
                        nc.scalar.dma_start(out=xt[:], in_=x0T[:, t * P:(t + 1) * P])
                    else:
                        nc.scalar.dma_start(out=xt[:], in_=xmT[l - 1][:, t * P:(t + 1) * P])
                    mm = nps_.tile([P, 2 * D], f32, tag="mm")
                    nc.tensor.matmul(out=mm[:], lhsT=xt[:], rhs=wlr[:], start=True, stop=True)
                    xlr = xrp.tile([P, 2 * D], bf16, tag=f"xlr{t}")
                    nc.scalar.activation(out=xlr[:], in_=mm[:], func=AF.Copy)
                    nc.sync.dma_start(out=xl_sh[t * P:(t + 1) * P, :], in_=xlr[:, 0:D])
                    xlr_all.append(xlr)
                    # first-half AllGather as soon as rows [0, S2) are stored
                    if use_collective and (t + 1) * P == S2:
                        nc.gpsimd.collective_compute(
                            "AllGather", OP.bypass,
                            replica_groups=[list(range(ncores))],
                            ins=[xl_sh[0:S2, :]], outs=[xl_full[0:Np // 2, :]],
                        )
                if use_collective:
                    nc.gpsimd.collective_compute(
                        "AllGather", OP.bypass,
                        replica_groups=[list(range(ncores))],
                        ins=[xl_sh[S2:Nshard, :]], outs=[xl_full[Np // 2:Np, :]],
                    )
                else:
                    nc.sync.dma_start(out=xl_full[:], in_=xl_sh[:])

                # ---- edge phase, gathers grouped G blocks at a time
                for gi, (blks, co) in enumerate(offs):
                    idst = gp.tile([P, gc], i32, tag="idst")
                    gcols = co[-1] + chks[blks[-1]]
                    nc.sync.dma_start(out=idst[:, 0:gcols], in_=ids[gi, :, 0:gcols])
                    xl_g = gp.tile([P, gc * D], bf16, tag="xls")
                    for ci in range(gcols):
                        nc.gpsimd.indirect_dma_start(
                            out=xl_g[:, ci * D:(ci + 1) * D],
                            out_offset=None, in_=xl_full[:],
                            in_offset=bass.IndirectOffsetOnAxis(
                                ap=idst[:, ci:ci + 1], axis=0))
                    for blk, coff in zip(blks, co):
                        chk = chks[blk]
                        xl_s = xl_g[:, coff * D:(coff + chk) * D]
                        oht = ep.tile([P, CH * 2 * P], bf16, tag="oht")
                        nc.scalar.dma_start(out=oht[:, 0:chk * 2 * P],
                                            in_=oh[blk, :, 0:chk * 2 * P])
                        xrb = xlr_all[blk]
                        # xr[dst] per chunk via one-hot matmul (PE, no DMA gather)
                        xr_ps = xps.tile([P, CH * D], f32, tag="xrps")
                        for ch in range(chk):
                            nc.tensor.matmul(
                                out=xr_ps[:, ch * D:(ch + 1) * D],
                                lhsT=oht[:, ch * 2 * P:ch * 2 * P + P],
                                rhs=xrb[:, D:2 * D], start=True, stop=True)
                        nd = chk * D
                        g = ep.tile([P, CH * D], bf16, tag="g")
                        nc.vector.tensor_tensor(out=g[:, 0:nd], in0=xl_s,
                                                in1=xr_ps[:, 0:nd], op=OP.add)
                        gl = ep.tile([P, CH * D], bf16, tag="gl")
                        # leaky_relu = max(0.2*g, g); must be DVE (ACT Lrelu drops alpha)
                        nc.vector.scalar_tensor_tensor(
                            out=gl[:, 0:nd], in0=g[:, 0:nd], scalar=NEG_SLOPE,
                            in1=g[:, 0:nd], op0=OP.mult, op1=OP.max)
                        ge = ep.tile([P, CH * D], bf16, tag="ge")
                        nc.vector.tensor_tensor(out=ge[:, 0:nd], in0=gl[:, 0:nd],
                                                in1=attb[:, 0:nd], op=OP.mult)
                        e = ep.tile([P, CH * H], f32, tag="e")
                        nc.vector.tensor_reduce(
                            out=e[:, 0:chk * H],
                            in_=ge[:, 0:nd].rearrange("p (sh c) -> p sh c", c=C),
                            axis=mybir.AxisListType.X, op=OP.add)
                        vals = ep.tile([P, CH * W], bf16, tag="vals")
                        vals_v = vals[:].rearrange("p (s w) -> p s w", s=CH)
                        nc.scalar.activation(
                            out=vals_v[:, 0:chk, D:W],
                            in_=e[:, 0:chk * H].rearrange("p (s h) -> p s h", s=chk),
                            func=AF.Exp)
                        nc.vector.tensor_tensor(
                            out=vals_v[:, 0:chk, 0:D].rearrange("p s (h c) -> p s h c", h=H),
                            in0=xl_s.rearrange("p (s h c) -> p s h c", s=chk, h=H),
                            in1=vals_v[:, 0:chk, D:W].unsqueeze(3).to_broadcast([P, chk, H, C]),
                            op=OP.mult)
                        ps = blkps.tile([P, W], f32, tag="ps")
                        for ch in range(chk):
                            nc.tensor.matmul(
                                out=ps[:], lhsT=oht[:, ch * 2 * P + P:(ch + 1) * 2 * P],
                                rhs=vals[:, ch * W:(ch + 1) * W],
                                start=(ch == 0), stop=(ch == chk - 1))
                        # ---- finalize: self-loop term, div by denom, +bias, elu
                        gs = fp.tile([P, D], bf16, tag="gs")
                        nc.vector.tensor_tensor(out=gs[:], in0=xrb[:, 0:D],
                                                in1=xrb[:, D:2 * D], op=OP.add)
                        nc.vector.scalar_tensor_tensor(
                            out=gs[:], in0=gs[:], scalar=NEG_SLOPE, in1=gs[:],
                            op0=OP.mult, op1=OP.max)
                        nc.vector.tensor_tensor(out=gs[:], in0=gs[:], in1=attb[:, 0:D],
                                                op=OP.mult)
                        es = fp.tile([P, H], f32, tag="es")
                        nc.vector.tensor_reduce(
                            out=es[:], in_=gs[:].rearrange("p (h c) -> p h c", h=H),
                            axis=mybir.AxisListType.X, op=OP.add)
                        pse = fp.tile([P, H], f32, tag="pse")
                        nc.scalar.activation(out=pse[:], in_=es[:], func=AF.Exp)
                        den = fp.tile([P, H], f32, tag="den")
                        nc.vector.tensor_tensor(out=den[:], in0=ps[:, D:W], in1=pse[:],
                                                op=OP.add)
                        r = fp.tile([P, H], f32, tag="r")
                        nc.vector.reciprocal(out=r[:], in_=den[:])
                        num = fp.tile([P, D], f32, tag="num")
                        nc.vector.tensor_tensor(
                            out=num[:].rearrange("p (h c) -> p h c", h=H),
                            in0=xrb[:, 0:D].rearrange("p (h c) -> p h c", h=H),
                            in1=pse[:].unsqueeze(2).to_broadcast([P, H, C]),
                            op=OP.mult)
                        nc.vector.tensor_tensor(out=num[:], in0=num[:], in1=ps[:, 0:D],
                                                op=OP.add)
                        o = fp.tile([P, D], f32, tag="o")
                        nc.vector.tensor_tensor(
                            out=o[:].rearrange("p (h c) -> p h c", h=H),
                            in0=num[:].rearrange("p (h c) -> p h c", h=H),
                            in1=r[:].unsqueeze(2).to_broadcast([P, H, C]),
                            op=OP.mult)
                        nc.vector.tensor_tensor(out=o[:], in0=o[:], in1=biasb[:], op=OP.add)
                        t1 = fp.tile([P, D], f32, tag="t1")
                        nc.vector.tensor_scalar(
                            out=t1[:], in0=o[:], scalar1=0.0, scalar2=None, op0=OP.min)
                        nc.scalar.activation(out=t1[:], in_=t1[:], func=AF.Exp)
                        # elu tail: o = max(o, t1 - 1) in one fused DVE op
                        nc.vector.scalar_tensor_tensor(
                            out=o[:], in0=t1[:], scalar=-1.0, in1=o[:],
                            op0=OP.add, op1=OP.max)
                        if l == L - 1:
                            nc.sync.dma_start(out=y[blk * P:(blk + 1) * P, :], in_=o[:])
                        else:
                            ob = fp.tile([P, D], bf16, tag="ob")
                            nc.vector.tensor_copy(out=ob[:], in_=o[:])
                            obT_ps = nps_.tile([D, P], bf16, tag="obT")
                            nc.tensor.transpose(out=obT_ps[:], in_=ob[:], identity=ident[:])
                            obT = fp.tile([D, P], bf16, tag="obTs")
                            nc.scalar.activation(out=obT[:], in_=obT_ps[:], func=AF.Copy)
                            nc.sync.dma_start(out=x_out[:, blk * P:(blk + 1) * P], in_=obT[:])
    nc.compile()
    return nc


# ---------------------------------------------------------------- entry

def make_inmaps(inputs, ncores):
    x = np.asarray(inputs['x'], np.float32)
    ei = np.asarray(inputs['edge_index'], np.int32)
    N, F = x.shape
    H, C = np.asarray(inputs['att0']).shape
    D = H * C
    L = 3
    # the synthetic self-loops (PyG add_self_loops) are handled analytically in
    # the kernel finalize; natural (i,i) edges stay in the edge stream
    src = ei[0]
    dst = ei[1]
    Nshard, nblk, chks, offs, gc, idsg, oh = prep_edges(src, dst, N, ncores)
    CH = max(chks)
    xp = np.zeros((Nshard * ncores, F), np.float32)
    xp[:N] = x
    ident = np.eye(P, dtype=BF16)
    dims_in = [F] + [D] * (L - 1)
    base = {"ident": ident}
    for l in range(L):
        wl = np.asarray(inputs[f'Wl{l}'], np.float32)
        wr = np.asarray(inputs[f'Wr{l}'], np.float32)
        base[f"Wlr{l}"] = np.concatenate([wl, wr], axis=1).astype(BF16)
        att = np.asarray(inputs[f'att{l}'], np.float32).reshape(1, D)
        base[f"attb{l}"] = np.broadcast_to(np.tile(att, (1, CH)), (P, CH * D)).astype(BF16)
        b = np.asarray(inputs[f'b{l}'], np.float32).reshape(1, D)
        base[f"biasb{l}"] = np.broadcast_to(b, (P, D)).copy()
    in_maps = []
    for c in range(ncores):
        mm = dict(base)
        mm["x0T"] = np.ascontiguousarray(
            xp[c * Nshard:(c + 1) * Nshard].T).astype(BF16)
        mm["ids"] = np.ascontiguousarray(idsg[c])
        mm["oh"] = np.ascontiguousarray(oh[c])
        in_maps.append(mm)
    return in_maps, Nshard, nblk, chks, offs, gc, dims_in, H, C, N, D


def kernel(**inputs):
    from concourse.bass_utils import run_bass_kernel_spmd
    ncores = 8
    (in_maps, Nshard, nblk, chks, offs, gc, dims_in, H, C, N, D) = \
        make_inmaps(inputs, ncores)
    nc = build_program(ncores, Nshard, nblk, chks, offs, gc, dims_in, H, C,
                       use_collective=True)
    res = run_bass_kernel_spmd(nc, in_maps, list(range(ncores)))
    out = np.concatenate([res.results[c]["y"] for c in range(ncores)], axis=0)
    return out[:N].astype(np.float32)


if __name__ == "__main__":
    pass
